# revision 4
# baseline (speedup 1.0000x reference)
"""DiT attention (B=2, S=2048, H=2048, 16 heads / 4 KV heads, RoPE) on 8 trn2
NeuronCores.

Sharding: core c -> batch b = c//4, head-group g = c%4 (q-heads 4g..4g+3 and
kv-head g).  Each core computes its heads' attention for its batch plus the
partial output projection over its 512 O-columns; the host sums the 4 partials
per batch and adds the output bias.

All matmuls run in bf16 (f32 PSUM accumulation).  Structure:
  1. KV pass: K/V projections + K RoPE + V transpose for all of S.
  2. Per 512-wide q-chunk qc: attention for the 4 heads, with the Q
     projection of chunk qc+1 and the output projection of chunk qc-1
     interleaved into the PE stream as fillers.
The softmax denominator is accumulated on the vector engine (bf16 adds of the
exp tiles) with a single ones-column matmul per (qc, head) instead of a
full PE matmul per key tile.  exp() runs on the scalar engine over
[128, 1024] PSUM groups (two 512-wide score tiles per instruction).
"""
import sys

if '/opt/trn_rl_repo' not in sys.path:
    sys.path.insert(0, '/opt/trn_rl_repo')

from contextlib import ExitStack

import numpy as np
import ml_dtypes

import concourse.bass as bass  # noqa: F401  (registers types)
import concourse.tile as tile
import concourse.mybir as mybir
from concourse import bacc, bass_utils

B, S, H = 2, 2048, 2048
NH, NKV, HD = 16, 4, 128
P = 128
SCALING = HD ** -0.5
KO = H // P          # 16 contraction tiles for the projections
NQH = NH // NKV      # 4 q heads per core
CH = 512             # chunk width (KV pass, q chunks, y chunks)
NCH = S // CH        # 4
KT = S // P          # 16 key tiles
F32 = mybir.dt.float32
BF16 = mybir.dt.bfloat16
AF = mybir.ActivationFunctionType

_NC_CACHE = []


def _build_nc():
    nc = bacc.Bacc("TRN2", target_bir_lowering=False, debug=False,
                   enable_asserts=True, num_devices=8)
    xt = nc.dram_tensor("xt", [H, S], BF16, kind="ExternalInput").ap()
    wq = nc.dram_tensor("wq", [H, NQH * HD], BF16, kind="ExternalInput").ap()
    wk = nc.dram_tensor("wk", [H, HD], BF16, kind="ExternalInput").ap()
    wv = nc.dram_tensor("wv", [H, HD], BF16, kind="ExternalInput").ap()
    wo = nc.dram_tensor("wo", [NQH * HD, H], BF16, kind="ExternalInput").ap()
    cosT = nc.dram_tensor("cosT", [HD, S], BF16, kind="ExternalInput").ap()
    sinT = nc.dram_tensor("sinT", [HD, S], BF16, kind="ExternalInput").ap()
    bqT = nc.dram_tensor("bqT", [HD, NQH], F32, kind="ExternalInput").ap()
    bkT = nc.dram_tensor("bkT", [HD, 1], F32, kind="ExternalInput").ap()
    bvT = nc.dram_tensor("bvT", [HD, 1], F32, kind="ExternalInput").ap()
    # [:, 0:128] identity (PE transpose), [:, 128] all-ones (denominator)
    ido = nc.dram_tensor("ido", [P, P + 1], BF16, kind="ExternalInput").ap()
    y = nc.dram_tensor("y", [S, H], F32, kind="ExternalOutput").ap()

    with tile.TileContext(nc) as tc, ExitStack() as ctx:
        const = ctx.enter_context(tc.tile_pool(name="const", bufs=1))
        wq_sb = const.tile([P, KO, NQH * HD], BF16)
        wk_sb = const.tile([P, KO, HD], BF16)
        wv_sb = const.tile([P, KO, HD], BF16)
        wo_sb = const.tile([P, NQH, H], BF16)
        cos_sb = const.tile([HD, S], BF16)
        sin_sb = const.tile([HD, S], BF16)
        bq_sb = const.tile([HD, NQH], F32)
        bk_sb = const.tile([HD, 1], F32)
        bv_sb = const.tile([HD, 1], F32)
        ido_sb = const.tile([P, P + 1], BF16)

        res = ctx.enter_context(tc.tile_pool(name="res", bufs=1))
        xt_sb = res.tile([P, KO, S], BF16)    # X^T resident, k-tiled
        qrop = res.tile([HD, NQH, S], BF16)   # Q^T roped; reused as O^T later
        krop = res.tile([HD, S], BF16)        # K^T roped
        v_sb = res.tile([P, KT, HD], BF16)    # V natural, k-tiled

        rope = ctx.enter_context(tc.tile_pool(name="rope", bufs=6))
        vtp = ctx.enter_context(tc.tile_pool(name="vtp", bufs=2))
        ptp = ctx.enter_context(tc.tile_pool(name="ptp", bufs=3))
        recp = ctx.enter_context(tc.tile_pool(name="recp", bufs=2))
        rbp = ctx.enter_context(tc.tile_pool(name="rbp", bufs=2))
        ysp = ctx.enter_context(tc.tile_pool(name="ysp", bufs=4))
        psS = ctx.enter_context(tc.tile_pool(name="psS", bufs=2, space="PSUM"))
        psO = ctx.enter_context(tc.tile_pool(name="psO", bufs=2, space="PSUM"))
        psM = ctx.enter_context(tc.tile_pool(name="psM", bufs=1, space="PSUM"))
        psSM = ctx.enter_context(
            tc.tile_pool(name="psSM", bufs=1, space="PSUM"))

        # ---------------- input DMA (ordering = startup latency) ----------
        # gpsimd (swdge) queue: small consts + tables + odd xt chunks
        nc.gpsimd.dma_start(bk_sb[:], bkT)
        nc.gpsimd.dma_start(bv_sb[:], bvT)
        nc.gpsimd.dma_start(bq_sb[:], bqT)
        nc.gpsimd.dma_start(ido_sb[:], ido)
        xt_r = xt.rearrange("(ko p) s -> p ko s", p=P)
        # sync queue: wk, wv, xt row 0, then wq/wo (needed ~15/~45us in);
        # xt rows are full-S (4KB/partition contiguous) spread over the
        # sync, scalar(hwdge) and gpsimd(swdge) queues for parallelism.
        nc.sync.dma_start(wk_sb[:], wk.rearrange("(ko p) m -> p ko m", p=P))
        nc.sync.dma_start(wv_sb[:], wv.rearrange("(ko p) m -> p ko m", p=P))
        nc.sync.dma_start(xt_sb[:, 0:1, :], xt_r[:, 0:1, :])
        nc.gpsimd.dma_start(cos_sb[:], cosT)
        nc.gpsimd.dma_start(sin_sb[:], sinT)
        for ko in range(1, KO):
            ksl = slice(ko, ko + 1)
            eng = nc.scalar if ko % 2 == 1 else nc.gpsimd
            eng.dma_start(xt_sb[:, ksl, :], xt_r[:, ksl, :])
        nc.sync.dma_start(wq_sb[:], wq.rearrange("(ko p) m -> p ko m", p=P))
        nc.sync.dma_start(wo_sb[:], wo.rearrange("(h p) n -> p h n", p=P))

        def rope_tail(ps, bias, dst, scol, on_act=True):
            """bias-add (PSUM->SBUF bf16), rotate-half swap (DMA),
            q*cos + rot(q)*sin (DVE) -> dst."""
            qf = rope.tile([HD, CH], BF16, tag="qf")
            if on_act:
                nc.scalar.activation(qf[:], ps[:], AF.Identity, bias=bias,
                                     scale=1.0)
            else:
                nc.vector.tensor_scalar_add(qf[:], ps[:], bias)
            qs = rope.tile([HD, CH], BF16, tag="qs")
            nc.sync.dma_start(qs[0:64, :], qf[64:128, :])
            nc.sync.dma_start(qs[64:128, :], qf[0:64, :])
            t2 = rope.tile([HD, CH], BF16, tag="t2")
            nc.vector.tensor_mul(t2[:], qs[:], sin_sb[:, scol])
            nc.vector.tensor_mul(qf[:], qf[:], cos_sb[:, scol])
            nc.vector.tensor_add(dst, qf[:], t2[:])

        # ---------------- KV pass: K/V proj + K RoPE + V transpose --------
        for c in range(NCH):
            scol = slice(c * CH, (c + 1) * CH)
            kps = psM.tile([HD, CH], F32, tag="m")
            for ko in range(KO):
                nc.tensor.matmul(kps[:], wk_sb[:, ko, :], xt_sb[:, ko, scol],
                                 start=(ko == 0), stop=(ko == KO - 1))
            rope_tail(kps, bk_sb[:, 0:1], krop[:, scol], scol)
            vps = psM.tile([HD, CH], F32, tag="m")
            for ko in range(KO):
                nc.tensor.matmul(vps[:], wv_sb[:, ko, :], xt_sb[:, ko, scol],
                                 start=(ko == 0), stop=(ko == KO - 1))
            vt = vtp.tile([HD, CH], BF16, tag="vt")
            nc.scalar.activation(vt[:], vps[:], AF.Identity,
                                 bias=bv_sb[:, 0:1], scale=1.0)
            for st in range(CH // P):
                kt = c * (CH // P) + st
                vtr = psM.tile([P, P], BF16, tag="m")
                nc.tensor.transpose(vtr[:], vt[:, st * P:(st + 1) * P],
                                    ido_sb[:, 0:P])
                nc.vector.tensor_copy(v_sb[:, kt, :], vtr[:])

        # ---------------- fillers ----------------------------------------
        def emit_qproj(qc, h):
            """Q projection + RoPE for one head of chunk qc."""
            scol = slice(qc * CH, (qc + 1) * CH)
            qps = psM.tile([HD, CH], F32, tag="m")
            for ko in range(KO):
                nc.tensor.matmul(qps[:],
                                 wq_sb[:, ko, h * HD:(h + 1) * HD],
                                 xt_sb[:, ko, scol],
                                 start=(ko == 0), stop=(ko == KO - 1))
            rope_tail(qps, bq_sb[:, h:h + 1], qrop[:, h, scol], scol,
                      on_act=False)

        def emit_p3(qc, qt, ycn):
            """output-projection partial for q-tile qt, y-columns ycn."""
            ysl = slice(ycn * CH, (ycn + 1) * CH)
            y_ps = psM.tile([P, CH], F32, tag="m")
            for hh in range(NQH):
                nc.tensor.matmul(y_ps[:],
                                 qrop[:, hh, qt * P:(qt + 1) * P],
                                 wo_sb[:, hh, ysl],
                                 start=(hh == 0), stop=(hh == NQH - 1))
            y_sb = ysp.tile([P, CH], F32, tag="ysb")
            nc.vector.tensor_copy(y_sb[:], y_ps[:])
            nc.sync.dma_start(y[qt * P:(qt + 1) * P, ysl], y_sb[:])

        # ---------------- attention with interleaved fillers --------------
        # Q projection for chunk 0 runs up front (nothing to hide it under).
        for h in range(NQH):
            emit_qproj(0, h)

        NG = KT // 2          # 8 exp groups of 2 key tiles per (qc, h)
        for qc in range(NCH):
            qsl = slice(qc * CH, (qc + 1) * CH)
            fillers = []
            if qc > 0:
                for qt in range((qc - 1) * (CH // P), qc * (CH // P)):
                    for ycn in range(NCH):
                        fillers.append(('p3', qc - 1, qt, ycn))
            if qc < NCH - 1:
                for h in range(NQH):
                    fillers.append(('qp', qc + 1, h))
            fi = [0]
            nslots = NQH * NG * 2
            stride = max(1, -(-nslots // max(1, len(fillers))))

            def drain(slot):
                if fi[0] < len(fillers) and slot % stride == 0:
                    f = fillers[fi[0]]
                    fi[0] += 1
                    if f[0] == 'p3':
                        emit_p3(f[1], f[2], f[3])
                    else:
                        emit_qproj(f[1], f[2])

            slot = [0]
            for h in range(NQH):
                o_ps = psO.tile([HD, CH], F32, tag="o")
                s_ps = psSM.tile([1, CH], F32, tag="sm")
                sgs = [None] * NG

                def scores(g):
                    sg = psS.tile([P, 2 * CH], F32, tag="sg")
                    sgs[g] = sg
                    for j in range(2):
                        nc.tensor.matmul(
                            sg[:, j * CH:(j + 1) * CH],
                            krop[:, (2 * g + j) * P:(2 * g + j + 1) * P],
                            qrop[:, h, qsl], start=True, stop=True)

                scores(0)
                for g in range(NG):
                    if g + 1 < NG:
                        scores(g + 1)
                    drain(slot[0]); slot[0] += 1
                    pt = ptp.tile([P, 2 * CH], BF16, tag="pt")
                    nc.scalar.activation(pt[:], sgs[g][:], AF.Exp,
                                         scale=SCALING)
                    sgs[g] = None
                    for j in range(2):
                        kt = 2 * g + j
                        nc.tensor.matmul(o_ps[:], v_sb[:, kt, :],
                                         pt[:, j * CH:(j + 1) * CH],
                                         start=(kt == 0), stop=(kt == KT - 1))
                        nc.tensor.matmul(s_ps[:], ido_sb[:, P:P + 1],
                                         pt[:, j * CH:(j + 1) * CH],
                                         start=(kt == 0), stop=(kt == KT - 1))
                    drain(slot[0]); slot[0] += 1
                rec = recp.tile([1, CH], F32, tag="rec")
                nc.vector.reciprocal_approx_fast(rec[:], s_ps[:])
                rb = rbp.tile([P, CH], F32, tag="rb")
                nc.gpsimd.partition_broadcast(rb[:], rec[:])
                # normalized O^T overwrites the spent Q^T slice
                nc.vector.tensor_mul(qrop[:, h, qsl], o_ps[:], rb[:])
            # drain any leftover fillers for this qc
            while fi[0] < len(fillers):
                f = fillers[fi[0]]
                fi[0] += 1
                if f[0] == 'p3':
                    emit_p3(f[1], f[2], f[3])
                else:
                    emit_qproj(f[1], f[2])

        # tail: output projection for the last q-chunk
        for qt in range((NCH - 1) * (CH // P), NCH * (CH // P)):
            for ycn in range(NCH):
                emit_p3(NCH - 1, qt, ycn)

    nc.compile()
    return nc


def _get_nc():
    if not _NC_CACHE:
        _NC_CACHE.append(_build_nc())
    return _NC_CACHE[0]


def kernel(**inputs) -> np.ndarray:
    BF = ml_dtypes.bfloat16
    hs = np.asarray(inputs["hidden_states"], np.float32)
    cos = np.asarray(inputs["cos"], np.float32)
    sin = np.asarray(inputs["sin"], np.float32)
    Wq = np.asarray(inputs["Wq"], np.float32)
    bq = np.asarray(inputs["bq"], np.float32)
    Wk = np.asarray(inputs["Wk"], np.float32)
    bk = np.asarray(inputs["bk"], np.float32)
    Wv = np.asarray(inputs["Wv"], np.float32)
    bv = np.asarray(inputs["bv"], np.float32)
    Wo = np.asarray(inputs["Wo"], np.float32)
    bo = np.asarray(inputs["bo"], np.float32)

    nc = _get_nc()

    XT = [np.ascontiguousarray(hs[b].T.astype(BF)) for b in range(B)]
    cosT = [np.ascontiguousarray(cos[b].T.astype(BF)) for b in range(B)]
    sinTs = []
    for b in range(B):
        st = np.ascontiguousarray(sin[b].T)
        st[0:64] = -st[0:64]          # fold rotate_half sign into the table
        sinTs.append(st.astype(BF))
    ido = np.zeros((P, P + 1), np.float32)
    ido[:, 0:P] = np.eye(P, dtype=np.float32)
    ido[:, P] = 1.0
    ido = ido.astype(BF)

    in_maps = []
    for c in range(8):
        b, g = c // 4, c % 4
        in_maps.append({
            "xt": XT[b],
            "wq": np.ascontiguousarray(
                Wq[:, g * NQH * HD:(g + 1) * NQH * HD].astype(BF)),
            "wk": np.ascontiguousarray(Wk[:, g * HD:(g + 1) * HD].astype(BF)),
            "wv": np.ascontiguousarray(Wv[:, g * HD:(g + 1) * HD].astype(BF)),
            "wo": np.ascontiguousarray(
                Wo[g * NQH * HD:(g + 1) * NQH * HD, :].astype(BF)),
            "cosT": cosT[b],
            "sinT": sinTs[b],
            "bqT": np.ascontiguousarray(
                bq[g * NQH * HD:(g + 1) * NQH * HD].reshape(NQH, HD).T),
            "bkT": np.ascontiguousarray(
                bk[g * HD:(g + 1) * HD].reshape(1, HD).T),
            "bvT": np.ascontiguousarray(
                bv[g * HD:(g + 1) * HD].reshape(1, HD).T),
            "ido": ido,
        })

    res = bass_utils.run_bass_kernel_spmd(nc, in_maps, core_ids=list(range(8)))

    out = np.empty((B, S, H), np.float32)
    for b in range(B):
        acc = res.results[4 * b]["y"].copy()
        for g in range(1, 4):
            acc += res.results[4 * b + g]["y"]
        out[b] = acc + bo[None, :]
    return out


# revision 5
# speedup vs baseline: 1.3902x; 1.3902x over previous
"""DiT attention (B=2, S=2048, H=2048, 16 heads / 4 KV heads, RoPE) on 8 trn2
NeuronCores.

Sharding: core c -> batch b = c//4, head-group g = c%4 (q-heads 4g..4g+3 and
kv-head g).  Each core computes its heads' attention for its batch plus the
partial output projection over its 512 O-columns; the host sums the 4 partials
per batch and adds the output bias.

All matmuls run in bf16 (f32 PSUM accumulation).  Structure:
  1. KV pass: K/V projections + K RoPE + V transpose for all of S.
  2. Per 512-wide q-chunk qc: attention for the 4 heads, with the Q
     projection of chunk qc+1 and the output projection of chunk qc-1
     interleaved into the PE stream as fillers.
The softmax denominator is accumulated on the vector engine (bf16 adds of the
exp tiles) with a single ones-column matmul per (qc, head) instead of a
full PE matmul per key tile.  exp() runs on the scalar engine over
[128, 1024] PSUM groups (two 512-wide score tiles per instruction).
"""
import sys

if '/opt/trn_rl_repo' not in sys.path:
    sys.path.insert(0, '/opt/trn_rl_repo')

from contextlib import ExitStack

import numpy as np
import ml_dtypes

import concourse.bass as bass  # noqa: F401  (registers types)
import concourse.tile as tile
import concourse.mybir as mybir
from concourse import bacc, bass_utils

B, S, H = 2, 2048, 2048
NH, NKV, HD = 16, 4, 128
P = 128
SCALING = HD ** -0.5
KO = H // P          # 16 contraction tiles for the projections
NQH = NH // NKV      # 4 q heads per core
CH = 512             # chunk width (KV pass, q chunks, y chunks)
NCH = S // CH        # 4
KT = S // P          # 16 key tiles
F32 = mybir.dt.float32
BF16 = mybir.dt.bfloat16
AF = mybir.ActivationFunctionType

_NC_CACHE = []


def _build_nc():
    nc = bacc.Bacc("TRN2", target_bir_lowering=False, debug=False,
                   enable_asserts=True, num_devices=8)
    xt = nc.dram_tensor("xt", [H, S], BF16, kind="ExternalInput").ap()
    wq = nc.dram_tensor("wq", [H, NQH * HD], BF16, kind="ExternalInput").ap()
    wk = nc.dram_tensor("wk", [H, HD], BF16, kind="ExternalInput").ap()
    wv = nc.dram_tensor("wv", [H, HD], BF16, kind="ExternalInput").ap()
    wo = nc.dram_tensor("wo", [NQH * HD, H], BF16, kind="ExternalInput").ap()
    cosT = nc.dram_tensor("cosT", [HD, S], BF16, kind="ExternalInput").ap()
    sinT = nc.dram_tensor("sinT", [HD, S], BF16, kind="ExternalInput").ap()
    bqT = nc.dram_tensor("bqT", [HD, NQH], F32, kind="ExternalInput").ap()
    bkT = nc.dram_tensor("bkT", [HD, 1], F32, kind="ExternalInput").ap()
    bvT = nc.dram_tensor("bvT", [HD, 1], F32, kind="ExternalInput").ap()
    # [:, 0:128] identity (PE transpose), [:, 128] all-ones (denominator)
    ido = nc.dram_tensor("ido", [P, P + 1], BF16, kind="ExternalInput").ap()
    y = nc.dram_tensor("y", [S, H], F32, kind="ExternalOutput").ap()

    with tile.TileContext(nc) as tc, ExitStack() as ctx:
        const = ctx.enter_context(tc.tile_pool(name="const", bufs=1))
        wq_sb = const.tile([P, KO, NQH * HD], BF16)
        wk_sb = const.tile([P, KO, HD], BF16)
        wv_sb = const.tile([P, KO, HD], BF16)
        wo_sb = const.tile([P, NQH, H], BF16)
        cos_sb = const.tile([HD, S], BF16)
        sin_sb = const.tile([HD, S], BF16)
        bq_sb = const.tile([HD, NQH], F32)
        bk_sb = const.tile([HD, 1], F32)
        bv_sb = const.tile([HD, 1], F32)
        ido_sb = const.tile([P, P + 1], BF16)

        res = ctx.enter_context(tc.tile_pool(name="res", bufs=1))
        xt_sb = res.tile([P, KO, S], BF16)    # X^T resident, k-tiled
        qrop = res.tile([HD, NQH, S], BF16)   # Q^T roped; reused as O^T later
        krop = res.tile([HD, S], BF16)        # K^T roped
        v_sb = res.tile([P, KT, HD], BF16)    # V natural, k-tiled

        rope = ctx.enter_context(tc.tile_pool(name="rope", bufs=6))
        vtp = ctx.enter_context(tc.tile_pool(name="vtp", bufs=2))
        ptp = ctx.enter_context(tc.tile_pool(name="ptp", bufs=3))
        accp = ctx.enter_context(tc.tile_pool(name="accp", bufs=2))
        recp = ctx.enter_context(tc.tile_pool(name="recp", bufs=2))
        rbp = ctx.enter_context(tc.tile_pool(name="rbp", bufs=2))
        ysp = ctx.enter_context(tc.tile_pool(name="ysp", bufs=4))
        psS = ctx.enter_context(tc.tile_pool(name="psS", bufs=2, space="PSUM"))
        psO = ctx.enter_context(tc.tile_pool(name="psO", bufs=2, space="PSUM"))
        psM = ctx.enter_context(tc.tile_pool(name="psM", bufs=2, space="PSUM"))

        # ---------------- input DMA (ordering = startup latency) ----------
        # gpsimd (swdge) queue: small consts + tables + odd xt chunks
        nc.gpsimd.dma_start(bk_sb[:], bkT)
        nc.gpsimd.dma_start(bv_sb[:], bvT)
        nc.gpsimd.dma_start(bq_sb[:], bqT)
        nc.gpsimd.dma_start(ido_sb[:], ido)
        xt_r = xt.rearrange("(ko p) s -> p ko s", p=P)
        # sync queue: wk, wv, xt row 0, then wq/wo (needed ~15/~45us in);
        # xt rows are full-S (4KB/partition contiguous) spread over the
        # sync, scalar(hwdge) and gpsimd(swdge) queues for parallelism.
        nc.sync.dma_start(wk_sb[:], wk.rearrange("(ko p) m -> p ko m", p=P))
        nc.sync.dma_start(wv_sb[:], wv.rearrange("(ko p) m -> p ko m", p=P))
        nc.sync.dma_start(xt_sb[:, 0:1, :], xt_r[:, 0:1, :])
        nc.gpsimd.dma_start(cos_sb[:], cosT)
        nc.gpsimd.dma_start(sin_sb[:], sinT)
        for ko in range(1, KO):
            ksl = slice(ko, ko + 1)
            eng = nc.scalar if ko % 2 == 1 else nc.gpsimd
            eng.dma_start(xt_sb[:, ksl, :], xt_r[:, ksl, :])
        nc.sync.dma_start(wq_sb[:], wq.rearrange("(ko p) m -> p ko m", p=P))
        nc.sync.dma_start(wo_sb[:], wo.rearrange("(h p) n -> p h n", p=P))

        def rope_tail(ps, bias, dst, scol, on_act=True):
            """bias-add (PSUM->SBUF bf16), rotate-half swap (DMA),
            q*cos + rot(q)*sin (DVE) -> dst."""
            qf = rope.tile([HD, CH], BF16, tag="qf")
            if on_act:
                nc.scalar.activation(qf[:], ps[:], AF.Identity, bias=bias,
                                     scale=1.0)
            else:
                nc.vector.tensor_scalar_add(qf[:], ps[:], bias)
            qs = rope.tile([HD, CH], BF16, tag="qs")
            nc.sync.dma_start(qs[0:64, :], qf[64:128, :])
            nc.sync.dma_start(qs[64:128, :], qf[0:64, :])
            t2 = rope.tile([HD, CH], BF16, tag="t2")
            nc.vector.tensor_mul(t2[:], qs[:], sin_sb[:, scol])
            nc.vector.tensor_mul(qf[:], qf[:], cos_sb[:, scol])
            nc.vector.tensor_add(dst, qf[:], t2[:])

        # ---------------- KV pass: K/V proj + K RoPE + V transpose --------
        for c in range(NCH):
            scol = slice(c * CH, (c + 1) * CH)
            kps = psM.tile([HD, CH], F32, tag="m")
            for ko in range(KO):
                nc.tensor.matmul(kps[:], wk_sb[:, ko, :], xt_sb[:, ko, scol],
                                 start=(ko == 0), stop=(ko == KO - 1))
            rope_tail(kps, bk_sb[:, 0:1], krop[:, scol], scol)
            vps = psM.tile([HD, CH], F32, tag="m")
            for ko in range(KO):
                nc.tensor.matmul(vps[:], wv_sb[:, ko, :], xt_sb[:, ko, scol],
                                 start=(ko == 0), stop=(ko == KO - 1))
            vt = vtp.tile([HD, CH], BF16, tag="vt")
            nc.scalar.activation(vt[:], vps[:], AF.Identity,
                                 bias=bv_sb[:, 0:1], scale=1.0)
            for st in range(CH // P):
                kt = c * (CH // P) + st
                vtr = psM.tile([P, P], BF16, tag="m")
                nc.tensor.transpose(vtr[:], vt[:, st * P:(st + 1) * P],
                                    ido_sb[:, 0:P])
                nc.vector.tensor_copy(v_sb[:, kt, :], vtr[:])

        # ---------------- fillers ----------------------------------------
        def emit_qproj(qc, h):
            """Q projection + RoPE for one head of chunk qc."""
            scol = slice(qc * CH, (qc + 1) * CH)
            qps = psM.tile([HD, CH], F32, tag="m")
            for ko in range(KO):
                nc.tensor.matmul(qps[:],
                                 wq_sb[:, ko, h * HD:(h + 1) * HD],
                                 xt_sb[:, ko, scol],
                                 start=(ko == 0), stop=(ko == KO - 1))
            rope_tail(qps, bq_sb[:, h:h + 1], qrop[:, h, scol], scol)

        ycopy_flip = [0]

        def emit_p3(qc, qt, ycn):
            """output-projection partial for q-tile qt, y-columns ycn."""
            ysl = slice(ycn * CH, (ycn + 1) * CH)
            y_ps = psM.tile([P, CH], F32, tag="m")
            for hh in range(NQH):
                nc.tensor.matmul(y_ps[:],
                                 qrop[:, hh, qt * P:(qt + 1) * P],
                                 wo_sb[:, hh, ysl],
                                 start=(hh == 0), stop=(hh == NQH - 1))
            y_sb = ysp.tile([P, CH], F32, tag="ysb")
            # alternate the PSUM->SBUF copy between ACT and DVE
            if ycopy_flip[0] % 2 == 0:
                nc.scalar.activation(y_sb[:], y_ps[:], AF.Copy)
            else:
                nc.vector.tensor_copy(y_sb[:], y_ps[:])
            ycopy_flip[0] += 1
            nc.sync.dma_start(y[qt * P:(qt + 1) * P, ysl], y_sb[:])

        # ---------------- attention with interleaved fillers --------------
        # Q projection for chunk 0 runs up front (nothing to hide it under).
        for h in range(NQH):
            emit_qproj(0, h)

        NG = KT // 2          # 8 exp groups of 2 key tiles per (qc, h)
        for qc in range(NCH):
            qsl = slice(qc * CH, (qc + 1) * CH)
            fillers = []
            if qc > 0:
                for qt in range((qc - 1) * (CH // P), qc * (CH // P)):
                    for ycn in range(NCH):
                        fillers.append(('p3', qc - 1, qt, ycn))
            if qc < NCH - 1:
                for h in range(NQH):
                    fillers.append(('qp', qc + 1, h))
            fi = [0]
            nslots = NQH * NG * 2
            stride = max(1, -(-nslots // max(1, len(fillers))))

            def drain(slot):
                if fi[0] < len(fillers) and slot % stride == 0:
                    f = fillers[fi[0]]
                    fi[0] += 1
                    if f[0] == 'p3':
                        emit_p3(f[1], f[2], f[3])
                    else:
                        emit_qproj(f[1], f[2])

            slot = [0]
            for h in range(NQH):
                o_ps = psO.tile([HD, CH], F32, tag="o")
                acc = accp.tile([P, 2 * CH], BF16, tag="acc")
                sgs = [None] * NG

                def scores(g):
                    sg = psS.tile([P, 2 * CH], F32, tag="sg")
                    sgs[g] = sg
                    for j in range(2):
                        nc.tensor.matmul(
                            sg[:, j * CH:(j + 1) * CH],
                            krop[:, (2 * g + j) * P:(2 * g + j + 1) * P],
                            qrop[:, h, qsl], start=True, stop=True)

                scores(0)
                for g in range(NG):
                    if g + 1 < NG:
                        scores(g + 1)
                    drain(slot[0]); slot[0] += 1
                    pt = ptp.tile([P, 2 * CH], BF16, tag="pt")
                    nc.scalar.activation(pt[:], sgs[g][:], AF.Exp,
                                         scale=SCALING)
                    sgs[g] = None
                    for j in range(2):
                        kt = 2 * g + j
                        nc.tensor.matmul(o_ps[:], v_sb[:, kt, :],
                                         pt[:, j * CH:(j + 1) * CH],
                                         start=(kt == 0), stop=(kt == KT - 1))
                    if g == 0:
                        nc.vector.tensor_copy(acc[:], pt[:])
                    else:
                        nc.vector.tensor_add(acc[:], acc[:], pt[:])
                    drain(slot[0]); slot[0] += 1
                s_ps = psM.tile([1, CH], F32, tag="m")
                nc.tensor.matmul(s_ps[:], ido_sb[:, P:P + 1], acc[:, 0:CH],
                                 start=True, stop=False)
                nc.tensor.matmul(s_ps[:], ido_sb[:, P:P + 1], acc[:, CH:2 * CH],
                                 start=False, stop=True)
                rec = recp.tile([1, CH], F32, tag="rec")
                nc.vector.reciprocal_approx_fast(rec[:], s_ps[:])
                rb = rbp.tile([P, CH], F32, tag="rb")
                nc.gpsimd.partition_broadcast(rb[:], rec[:])
                # normalized O^T overwrites the spent Q^T slice
                nc.vector.tensor_mul(qrop[:, h, qsl], o_ps[:], rb[:])
            # drain any leftover fillers for this qc
            while fi[0] < len(fillers):
                f = fillers[fi[0]]
                fi[0] += 1
                if f[0] == 'p3':
                    emit_p3(f[1], f[2], f[3])
                else:
                    emit_qproj(f[1], f[2])

        # tail: output projection for the last q-chunk
        for qt in range((NCH - 1) * (CH // P), NCH * (CH // P)):
            for ycn in range(NCH):
                emit_p3(NCH - 1, qt, ycn)

    nc.compile()
    return nc


def _get_nc():
    if not _NC_CACHE:
        _NC_CACHE.append(_build_nc())
    return _NC_CACHE[0]


def kernel(**inputs) -> np.ndarray:
    BF = ml_dtypes.bfloat16
    hs = np.asarray(inputs["hidden_states"], np.float32)
    cos = np.asarray(inputs["cos"], np.float32)
    sin = np.asarray(inputs["sin"], np.float32)
    Wq = np.asarray(inputs["Wq"], np.float32)
    bq = np.asarray(inputs["bq"], np.float32)
    Wk = np.asarray(inputs["Wk"], np.float32)
    bk = np.asarray(inputs["bk"], np.float32)
    Wv = np.asarray(inputs["Wv"], np.float32)
    bv = np.asarray(inputs["bv"], np.float32)
    Wo = np.asarray(inputs["Wo"], np.float32)
    bo = np.asarray(inputs["bo"], np.float32)

    nc = _get_nc()

    XT = [np.ascontiguousarray(hs[b].T.astype(BF)) for b in range(B)]
    cosT = [np.ascontiguousarray(cos[b].T.astype(BF)) for b in range(B)]
    sinTs = []
    for b in range(B):
        st = np.ascontiguousarray(sin[b].T)
        st[0:64] = -st[0:64]          # fold rotate_half sign into the table
        sinTs.append(st.astype(BF))
    ido = np.zeros((P, P + 1), np.float32)
    ido[:, 0:P] = np.eye(P, dtype=np.float32)
    ido[:, P] = 1.0
    ido = ido.astype(BF)

    in_maps = []
    for c in range(8):
        b, g = c // 4, c % 4
        in_maps.append({
            "xt": XT[b],
            "wq": np.ascontiguousarray(
                Wq[:, g * NQH * HD:(g + 1) * NQH * HD].astype(BF)),
            "wk": np.ascontiguousarray(Wk[:, g * HD:(g + 1) * HD].astype(BF)),
            "wv": np.ascontiguousarray(Wv[:, g * HD:(g + 1) * HD].astype(BF)),
            "wo": np.ascontiguousarray(
                Wo[g * NQH * HD:(g + 1) * NQH * HD, :].astype(BF)),
            "cosT": cosT[b],
            "sinT": sinTs[b],
            "bqT": np.ascontiguousarray(
                bq[g * NQH * HD:(g + 1) * NQH * HD].reshape(NQH, HD).T),
            "bkT": np.ascontiguousarray(
                bk[g * HD:(g + 1) * HD].reshape(1, HD).T),
            "bvT": np.ascontiguousarray(
                bv[g * HD:(g + 1) * HD].reshape(1, HD).T),
            "ido": ido,
        })

    res = bass_utils.run_bass_kernel_spmd(nc, in_maps, core_ids=list(range(8)))

    out = np.empty((B, S, H), np.float32)
    for b in range(B):
        acc = res.results[4 * b]["y"].copy()
        for g in range(1, 4):
            acc += res.results[4 * b + g]["y"]
        out[b] = acc + bo[None, :]
    return out
